# revision 1
# baseline (speedup 1.0000x reference)
"""Class-balanced softmax cross-entropy loss on 8 Trainium2 NeuronCores.

Math (per the reference nn.Module):
  counts N_c   = histogram of target over classes (whole batch)
  weights w_c  = (1-beta)/(1-beta^N_c), 0 where N_c == 0
  logp         = log_softmax(logits, axis=C)
  loss         = -sum_pix w[t] * logp[t_pix] / sum_pix w[t]

Decomposition used here: per core (data-parallel over batch B=8, one batch
item per core) compute per-class partials
  N_c = sum(target == c)
  A_c = sum_{target==c} logits[c]
  B_c = sum_{target==c} lse          (lse = log(sum_c' exp(logits[c'])))
Then on host: N = sum_cores N_c; w from N;
  loss = -(w . (A - B)) / (w . N)
No collectives needed; each core returns 3*19 floats.
"""

import numpy as np
from contextlib import ExitStack
import sys

for _p in ("/opt/trn_rl_repo",):
    if _p not in sys.path:
        sys.path.insert(0, _p)

from concourse import bass, mybir, tile
from concourse.bass_utils import run_bass_kernel_spmd

NCLASS = 19
BETA = 0.999
NCORES = 8
HW = 512 * 1024          # pixels per batch item (= per core)
P = 128                  # SBUF partitions
COLS = HW // P           # 4096
F = 512                  # free-dim chunk
NCHUNK = COLS // F       # 8

f32 = mybir.dt.float32
i32 = mybir.dt.int32
AF = mybir.ActivationFunctionType
ALU = mybir.AluOpType

# accumulator column layout: [A (NCHUNK*NCLASS) | B (...) | N (...)]
SEC = NCHUNK * NCLASS          # 152
ACC_COLS = 3 * SEC             # 456


def _build():
    """Raw-bass pipeline with manual semaphores.

    Engine roles per chunk k (buffer half h=k%2):
      ACT: issue X/T DMAs, exp x19, log; DVE: reduce(sumexp), 57 stt/ts
      accumulations; PE: final partition-reduce matmul.
    Cross-engine edges via explicit wait_ge/then_inc; within-engine order is
    program order. Transitive implications (exp done => X landed) are used
    to keep the wait count low.
    """
    nc = bass.Bass()
    logits = nc.declare_dram_parameter("logits", [NCLASS, P, COLS], f32, isOutput=False)
    target = nc.declare_dram_parameter("target", [P, COLS], i32, isOutput=False)
    out = nc.declare_dram_parameter("out", [1, ACC_COLS], f32, isOutput=True)

    EF = NCLASS * F
    X2 = nc.alloc_sbuf_tensor("X2", [P, 2 * EF], f32)
    E2 = nc.alloc_sbuf_tensor("E2", [P, 2 * EF], f32)
    Ti2 = nc.alloc_sbuf_tensor("Ti2", [P, 2 * F], i32)
    Tf2 = nc.alloc_sbuf_tensor("Tf2", [P, 2 * F], f32)
    S2 = nc.alloc_sbuf_tensor("S2", [P, 2 * F], f32)
    L2 = nc.alloc_sbuf_tensor("L2", [P, 2 * F], f32)
    junk = nc.alloc_sbuf_tensor("junk", [P, F], f32)
    ABN = nc.alloc_sbuf_tensor("ABN", [P, ACC_COLS], f32)
    ones = nc.alloc_sbuf_tensor("ones", [P, 1], f32)
    ones_f = nc.alloc_sbuf_tensor("ones_f", [P, F], f32)
    res = nc.alloc_sbuf_tensor("res", [1, ACC_COLS], f32)
    ps = nc.alloc_psum_tensor("ps", [1, ACC_COLS], f32)

    with (
        nc.Block() as block,
        nc.semaphore("sem_x") as sem_x,
        nc.semaphore("sem_t") as sem_t,
        nc.semaphore("sem_exp") as sem_exp,
        nc.semaphore("sem_red") as sem_red,
        nc.semaphore("sem_log") as sem_log,
        nc.semaphore("sem_done") as sem_done,
        nc.semaphore("sem_mm") as sem_mm,
        nc.semaphore("sem_out") as sem_out,
    ):
        @block.scalar
        def _(act):
            for k in range(NCHUNK):
                h = k % 2
                if k >= 2:
                    act.wait_ge(sem_done, k - 1)   # bufs of chunk k-2 free
                act.dma_start(
                    X2[:, h * EF:(h + 1) * EF].rearrange("p (c f) -> p c f", c=NCLASS),
                    logits[:, :, k * F:(k + 1) * F].rearrange("c p f -> p c f"),
                ).then_inc(sem_x, 16)
                act.dma_start(
                    Ti2[:, h * F:(h + 1) * F], target[:, k * F:(k + 1) * F],
                ).then_inc(sem_t, 16)
                act.wait_ge(sem_x, 16 * (k + 1))
                for c in range(NCLASS):
                    ins = act.activation(
                        E2[:, h * EF + c * F: h * EF + (c + 1) * F],
                        X2[:, h * EF + c * F: h * EF + (c + 1) * F], AF.Exp)
                    if c == NCLASS - 1:
                        ins.then_inc(sem_exp, 1)
                act.wait_ge(sem_red, k + 1)
                act.activation(
                    L2[:, h * F:(h + 1) * F], S2[:, h * F:(h + 1) * F], AF.Ln,
                ).then_inc(sem_log, 1)
            # tail: psum -> sbuf -> dram
            act.wait_ge(sem_mm, 1)
            act.copy(res[:], ps[:])
            act.dma_start(out[:, :], res[:]).then_inc(sem_out, 16)
            act.wait_ge(sem_out, 16)

        @block.vector
        def _(dve):
            dve.memset(ABN[:], 0.0)
            dve.memset(ones[:], 1.0)
            dve.memset(ones_f[:], 1.0)
            for k in range(NCHUNK):
                h = k % 2
                dve.wait_ge(sem_exp, k + 1)   # E ready (implies X landed)
                dve.tensor_reduce(
                    S2[:, h * F:(h + 1) * F],
                    E2[:, h * EF:(h + 1) * EF].rearrange("p (c f) -> p f c", c=NCLASS),
                    axis=mybir.AxisListType.X, op=ALU.add,
                ).then_inc(sem_red, 1)
                dve.wait_ge(sem_t, 16 * (k + 1))
                Ti = Tf2[:, h * F:(h + 1) * F]
                dve.tensor_copy(Ti[:], Ti2[:, h * F:(h + 1) * F])
                for c in range(NCLASS):
                    dve.scalar_tensor_tensor(
                        out=junk[:], in0=Ti[:], scalar=float(c),
                        in1=X2[:, h * EF + c * F: h * EF + (c + 1) * F],
                        op0=ALU.is_equal, op1=ALU.mult,
                        accum_out=ABN[:, 0 * SEC + k * NCLASS + c: 0 * SEC + k * NCLASS + c + 1])
                dve.wait_ge(sem_log, k + 1)
                LSE = L2[:, h * F:(h + 1) * F]
                for c in range(NCLASS):
                    dve.scalar_tensor_tensor(
                        out=junk[:], in0=Ti[:], scalar=float(c), in1=LSE[:],
                        op0=ALU.is_equal, op1=ALU.mult,
                        accum_out=ABN[:, 1 * SEC + k * NCLASS + c: 1 * SEC + k * NCLASS + c + 1])
                for c in range(NCLASS):
                    # counts: single-src tensor_scalar runs in 2x_2P mode;
                    # op1 is the accum reduce op (add)
                    ins = dve.tensor_scalar(
                        out=junk[:], in0=Ti[:], scalar1=float(c), scalar2=None,
                        op0=ALU.is_equal, op1=ALU.add,
                        accum_out=ABN[:, 2 * SEC + k * NCLASS + c: 2 * SEC + k * NCLASS + c + 1])
                    if c == NCLASS - 1:
                        ins.then_inc(sem_done, 1)

        @block.tensor
        def _(pe):
            pe.wait_ge(sem_done, NCHUNK)
            pe.matmul(ps[:], lhsT=ones[:], rhs=ABN[:], start=True, stop=True).then_inc(sem_mm, 1)

    return nc


def _build_tile_unused():
    nc = bass.Bass()
    logits = nc.declare_dram_parameter("logits", [NCLASS, P, COLS], f32, isOutput=False)
    target = nc.declare_dram_parameter("target", [P, COLS], i32, isOutput=False)
    out = nc.declare_dram_parameter("out", [1, ACC_COLS], f32, isOutput=True)

    with ExitStack() as ctx:
        tc = ctx.enter_context(tile.TileContext(nc))
        xpool = ctx.enter_context(tc.tile_pool(name="x", bufs=2))
        tpool = ctx.enter_context(tc.tile_pool(name="t", bufs=2))
        accpool = ctx.enter_context(tc.tile_pool(name="acc", bufs=1))
        pspool = ctx.enter_context(tc.tile_pool(name="ps", bufs=1, space="PSUM"))

        EF = NCLASS * F
        ABN = accpool.tile([P, ACC_COLS], f32)
        nc.vector.memset(ABN[:], 0.0)
        ones = accpool.tile([P, 1], f32)
        nc.vector.memset(ones[:], 1.0)
        # persistent manually double-buffered scratch (avoids Tile pool
        # release-waits, which overflow the 1-sync-wait ISA limit)
        Ebuf = accpool.tile([P, 2 * EF], f32)
        Sbuf = accpool.tile([P, 2 * F], f32)
        Lbuf = accpool.tile([P, 2 * F], f32)
        junk = accpool.tile([P, F], f32)
        pabs = accpool.tile([P, 1], f32)   # DVE absorber dst
        pdve = accpool.tile([P, 1], f32)   # DVE->ACT probe src
        pscr = accpool.tile([P, 1], f32)   # ACT probe dst

        probes = {}
        for k in range(NCHUNK):
            h = k % 2
            X = xpool.tile([P, EF], f32, tag="x")
            xdma = nc.scalar.dma_start(
                X[:].rearrange("p (c f) -> p c f", c=NCLASS),
                logits[:, :, k * F:(k + 1) * F].rearrange("c p f -> p c f"))
            Ti = tpool.tile([P, F], i32, tag="ti")
            tdma = nc.scalar.dma_start(Ti[:], target[:, k * F:(k + 1) * F])
            if k >= 2:
                # Order this chunk's DMAs after the probe that made ACT
                # observe DVE's consumption of the recycled buffers, so the
                # DMACopy needs no extra sync-wait (1-wait ISA limit).
                tile.add_dep_helper(xdma.ins, probes[k - 2], reason="recycle absorb")
                tile.add_dep_helper(tdma.ins, probes[k - 2], reason="recycle absorb")

            E = Ebuf[:, h * EF:(h + 1) * EF]
            for c in range(NCLASS):
                nc.scalar.activation(E[:, c * F:(c + 1) * F], X[:, c * F:(c + 1) * F], AF.Exp)

            S = Sbuf[:, h * F:(h + 1) * F]
            nc.vector.tensor_reduce(
                S[:], E[:].rearrange("p (c f) -> p f c", c=NCLASS),
                axis=mybir.AxisListType.X, op=ALU.add)
            LSE = Lbuf[:, h * F:(h + 1) * F]
            log_ins = nc.scalar.activation(LSE[:], S[:], AF.Ln).ins

            # Drain instructions accept many sync-waits; use one as the
            # absorber for ALL of this chunk's cross-engine edges so every
            # following DVE instruction needs at most its self-wait.
            dr = nc.vector.drain()
            tile.add_dep_helper(dr.ins, xdma.ins, reason="absorb x dma")
            tile.add_dep_helper(dr.ins, tdma.ins, reason="absorb t dma")
            tile.add_dep_helper(dr.ins, log_ins, reason="absorb log")
            for c in range(NCLASS):
                # A_c partial: sum over free of (T==c)*logits_c
                stt = nc.vector.scalar_tensor_tensor(
                    out=junk[:], in0=Ti[:], scalar=float(c), in1=X[:, c * F:(c + 1) * F],
                    op0=ALU.is_equal, op1=ALU.mult,
                    accum_out=ABN[:, 0 * SEC + k * NCLASS + c: 0 * SEC + k * NCLASS + c + 1])
                if c == 0:
                    # force the drain ahead of the whole stt block (ordered
                    # among themselves by the junk WAW chain)
                    tile.add_dep_helper(stt.ins, dr.ins, reason="stt after drain")
            for c in range(NCLASS):
                # B_c partial: sum over free of (T==c)*lse
                nc.vector.scalar_tensor_tensor(
                    out=junk[:], in0=Ti[:], scalar=float(c), in1=LSE[:],
                    op0=ALU.is_equal, op1=ALU.mult,
                    accum_out=ABN[:, 1 * SEC + k * NCLASS + c: 1 * SEC + k * NCLASS + c + 1])
            for c in range(NCLASS):
                # N_c partial: sum over free of (T==c)
                nc.vector.tensor_scalar(
                    out=junk[:], in0=Ti[:], scalar1=float(c), scalar2=1.0,
                    op0=ALU.is_equal, op1=ALU.mult,
                    accum_out=ABN[:, 2 * SEC + k * NCLASS + c: 2 * SEC + k * NCLASS + c + 1])
            nc.vector.tensor_copy(pdve[:], junk[:, 0:1])
            probes[k] = nc.scalar.copy(pscr[:], pdve[:]).ins

        ps = pspool.tile([1, ACC_COLS], f32)
        mm = nc.tensor.matmul(ps[:], lhsT=ones[:], rhs=ABN[:], start=True, stop=True)
        dr2 = nc.scalar.drain()
        tile.add_dep_helper(dr2.ins, mm.ins, reason="absorb matmul")
        res = accpool.tile([1, ACC_COLS], f32)
        nc.scalar.copy(res[:], ps[:])
        nc.scalar.dma_start(out[:, :], res[:])

    return nc


_CACHE = {}


def _get_nc():
    if "nc" not in _CACHE:
        _CACHE["nc"] = _build()
    return _CACHE["nc"]


def _run(logits, target, trace=False):
    nc = _get_nc()
    in_maps = []
    for i in range(NCORES):
        in_maps.append({
            "logits": np.ascontiguousarray(logits[i].reshape(NCLASS, P, COLS)),
            "target": np.ascontiguousarray(target[i].reshape(P, COLS)),
        })
    r = run_bass_kernel_spmd(nc, in_maps, core_ids=list(range(NCORES)), trace=trace)
    return r


def _combine(results):
    A = np.zeros(NCLASS, np.float64)
    B = np.zeros(NCLASS, np.float64)
    N = np.zeros(NCLASS, np.float64)
    for i in range(NCORES):
        r = results[i]["out"].astype(np.float64).reshape(3, NCHUNK, NCLASS).sum(axis=1)
        A += r[0]
        B += r[1]
        N += r[2]
    w = np.where(N > 0, (1.0 - BETA) / (1.0 - BETA ** N), 0.0)
    num = float((w * (A - B)).sum())
    den = float((w * N).sum())
    return np.float32(-num / den)


def kernel(logits, target):
    assert logits.shape == (NCORES, NCLASS, 512, 1024) and logits.dtype == np.float32
    assert target.shape == (NCORES, 512, 1024) and target.dtype == np.int32
    r = _run(logits, target, trace=False)
    return _combine(r.results)



# revision 5
# speedup vs baseline: 1.1726x; 1.1726x over previous
"""Class-balanced softmax cross-entropy loss on 8 Trainium2 NeuronCores.

Math: loss = -(sum_c w_c (A_c - B_c)) / (sum_c w_c N_c) with
  N_c = #{t==c}, A_c = sum_{t==c} X_c, B_c = sum_{t==c} lse,
  w_c = (1-beta)/(1-beta^N_c), lse = log sum_c' exp(X_c').

Per core (data-parallel over batch, one item per core):
  - exp -> bf16 on ACT; sumexp via bf16 tensor_tensor tree adds (2x mode)
  - lse = Ln(S) on ACT
  - B_c/N_c via "max-threshold families" on u_g = (64t+32-320g) - lse in
    fp16: Q[g,c'] = sum_f max(u_g, 64c') and M_c = sum_f (u_0 >= 64c),
    accumulated with 4x-mode tensor_scalar; host solves the triangular
    system (telescoping differences) for B and N.
  - A_c via scalar_tensor_tensor masked accumulation, split DVE/GPSIMD.
Host combines per-class partials from all 8 cores and applies weights.
"""

import numpy as np
import sys

for _p in ("/opt/trn_rl_repo",):
    if _p not in sys.path:
        sys.path.insert(0, _p)

from concourse import bass, mybir
from concourse.bass_utils import run_bass_kernel_spmd

NCLASS = 19
BETA = 0.999
NCORES = 8
P = 128
COLS = 4096
F = 1024
NCHUNK = COLS // F           # 4
NPAIR = NCHUNK // 2          # 2
NSLOT = 6                    # X class-slot ring
NDVE_A = 19                  # all A-stt on DVE (Pool rejects accum ops)
NGP_A = NCLASS - NDVE_A      # classes 5..18 on GPSIMD
GROUPS = [(0, 5), (5, 5), (10, 5), (15, 4)]   # (class0, size)
NQ = sum(sz + 1 for _, sz in GROUPS)          # 23 Q members
# accumulator column layout
A_OFF = 0                     # NCHUNK*19 = 76
Q_OFF = A_OFF + NCHUNK * NCLASS
M_OFF = Q_OFF + NPAIR * NQ    # 76+46 = 122
NCOL = M_OFF + NPAIR * NCLASS  # 160

f32 = mybir.dt.float32
bf16 = mybir.dt.bfloat16
f16 = mybir.dt.float16
i32 = mybir.dt.int32
AF = mybir.ActivationFunctionType
ALU = mybir.AluOpType


def _build():
    nc = bass.Bass()
    logits = nc.declare_dram_parameter("logits", [NCLASS, P, COLS], f32, isOutput=False)
    target = nc.declare_dram_parameter("target", [P, COLS], i32, isOutput=False)
    out = nc.declare_dram_parameter("out", [1, NCOL], f32, isOutput=True)

    Xr = nc.alloc_sbuf_tensor("Xr", [P, NSLOT * F], f32)      # class-slot ring
    E2 = nc.alloc_sbuf_tensor("E2", [P, 2 * NCLASS * F], bf16)
    Ti = nc.alloc_sbuf_tensor("Ti", [P, COLS], i32)
    Tb2 = nc.alloc_sbuf_tensor("Tb2", [P, 2 * F], bf16)
    SCR = nc.alloc_sbuf_tensor("SCR", [P, 17 * F], bf16)      # sumexp tree scratch
    S2 = nc.alloc_sbuf_tensor("S2", [P, 2 * F], bf16)
    L2 = nc.alloc_sbuf_tensor("L2", [P, 2 * F], bf16)         # lse
    T64 = nc.alloc_sbuf_tensor("T64", [P, F], bf16)
    U4 = nc.alloc_sbuf_tensor("U4", [P, 4 * 2 * F], f16)      # per-group pair u
    GPJ = nc.alloc_sbuf_tensor("GPJ", [P, F], f32)
    ACC = nc.alloc_sbuf_tensor("ACC", [P, NCOL], f32)
    ones = nc.alloc_sbuf_tensor("ones", [P, 1], f32)
    res = nc.alloc_sbuf_tensor("res", [1, NCOL], f32)
    ps = nc.alloc_psum_tensor("ps", [1, NCOL], f32)

    EF = NCLASS * F

    def slot(n):
        return Xr[:, (n % NSLOT) * F:(n % NSLOT + 1) * F]

    with (
        nc.Block() as block,
        nc.semaphore("s_x") as s_x,        # X class dmas landed (16 per)
        nc.semaphore("s_t") as s_t,        # Ti landed
        nc.semaphore("s_exp") as s_exp,    # exps done (1 per class-slot)
        nc.semaphore("s_S") as s_S,        # sumexp per chunk
        nc.semaphore("s_lse") as s_lse,    # ln per chunk
        nc.semaphore("s_u") as s_u,        # u-tt per chunk (4 per)
        nc.semaphore("s_aD") as s_aD,      # DVE A-stt count
        nc.semaphore("s_aG") as s_aG,      # GP A-stt count
        nc.semaphore("s_tb") as s_tb,      # Tb per chunk
        nc.semaphore("s_qm") as s_qm,      # Q/M pair blocks
        nc.semaphore("s_mm") as s_mm,
        nc.semaphore("s_out") as s_out,
    ):
        @block.sync
        def _(sp):
            for n in range(NCHUNK * NCLASS):
                k, c = n // NCLASS, n % NCLASS
                prev = n - NSLOT
                if prev >= 0:
                    kp, cp = prev // NCLASS, prev % NCLASS
                    sp.wait_ge(s_exp, prev + 1)
                    sp.wait_ge(s_aD, kp * NDVE_A + cp + 1)
                sp.dma_start(slot(n), logits[c, :, k * F:(k + 1) * F]).then_inc(s_x, 16)
            sp.wait_ge(s_out, 1)
            sp.dma_start(out[:, :], res[:]).then_inc(s_out, 16)
            sp.wait_ge(s_out, 17)

        @block.scalar
        def _(act):
            for k in range(NCHUNK):
                h = k % 2
                if k >= 2:
                    act.wait_ge(s_S, k - 1)   # E_h free after sumexp of k-2
                for c in range(NCLASS):
                    n = k * NCLASS + c
                    act.wait_ge(s_x, 16 * (n + 1))
                    ins = act.activation(
                        E2[:, h * EF + c * F: h * EF + (c + 1) * F], slot(n), AF.Exp)
                    ins.then_inc(s_exp, 1)
                # ln(S) -> lse
                act.wait_ge(s_S, k + 1)
                if k >= 2:
                    act.wait_ge(s_u, 4 * (k - 1))  # L_h free after u-tts of k-2
                act.activation(
                    L2[:, h * F:(h + 1) * F], S2[:, h * F:(h + 1) * F], AF.Ln,
                ).then_inc(s_lse, 1)
            act.wait_ge(s_mm, 1)
            act.copy(res[:], ps[:]).then_inc(s_out, 1)

        @block.vector
        def _(dve):
            dve.memset(ACC[:], 0.0)
            dve.memset(ones[:], 1.0)
            for k in range(NCHUNK):
                h = k % 2
                # ---- A-pass first: consume X slots as they land ----
                dve.wait_ge(s_tb, k + 1)
                for c in range(NDVE_A):
                    n = k * NCLASS + c
                    dve.wait_ge(s_x, 16 * (n + 1))
                    dve.scalar_tensor_tensor(
                        out=SCR[:, 0:2 * F].bitcast(f32), in0=Tb2[:, h * F:(h + 1) * F], scalar=float(c),
                        in1=slot(n), op0=ALU.is_equal, op1=ALU.mult,
                        accum_out=ACC[:, A_OFF + n:A_OFF + n + 1],
                    ).then_inc(s_aD, 1)
                # ---- sumexp tree over E_h [P, (c f)] ----
                dve.wait_ge(s_exp, NCLASS * (k + 1))
                if k >= 2:
                    dve.wait_ge(s_lse, k - 1)  # S_h free after ln of k-2
                Ev = E2[:, h * EF:(h + 1) * EF].rearrange("p (c f) -> p c f", c=NCLASS)
                # lvl1: 9 pairs of classes 0..17 -> SCR[0:9]
                dve.tensor_tensor(
                    out=SCR[:, 0:9 * F].rearrange("p (b f) -> p b f", b=9),
                    in0=Ev[:, 0:18, :].rearrange("p (b two) f -> p b two f", two=2)[:, :, 0, :],
                    in1=Ev[:, 0:18, :].rearrange("p (b two) f -> p b two f", two=2)[:, :, 1, :],
                    op=ALU.add)
                sv = SCR[:].rearrange("p (b f) -> p b f", b=17)
                # lvl2: 4 pairs from SCR[0:8] -> SCR[9:13]; SCR[8]+E18 -> SCR[13]
                dve.tensor_tensor(
                    out=sv[:, 9:13, :],
                    in0=sv[:, 0:8, :].rearrange("p (b two) f -> p b two f", two=2)[:, :, 0, :],
                    in1=sv[:, 0:8, :].rearrange("p (b two) f -> p b two f", two=2)[:, :, 1, :],
                    op=ALU.add)
                dve.tensor_tensor(out=sv[:, 13, :], in0=sv[:, 8, :], in1=Ev[:, 18, :], op=ALU.add)
                # lvl3: pairs from SCR[9:13] -> SCR[14:16]
                dve.tensor_tensor(
                    out=sv[:, 14:16, :],
                    in0=sv[:, 9:13, :].rearrange("p (b two) f -> p b two f", two=2)[:, :, 0, :],
                    in1=sv[:, 9:13, :].rearrange("p (b two) f -> p b two f", two=2)[:, :, 1, :],
                    op=ALU.add)
                # lvl4: SCR14+SCR15 -> SCR16 ; lvl5: S = SCR16 + SCR13
                dve.tensor_tensor(out=sv[:, 16, :], in0=sv[:, 14, :], in1=sv[:, 15, :], op=ALU.add)
                with nc.allow_low_precision("bf16 sumexp"):
                    dve.tensor_tensor(
                        out=S2[:, h * F:(h + 1) * F], in0=sv[:, 16, :], in1=sv[:, 13, :],
                        op=ALU.add).then_inc(s_S, 1)
                # ---- u_g build ----
                dve.wait_ge(s_lse, k + 1)
                for g, (c0, sz) in enumerate(GROUPS):
                    dve.tensor_scalar(
                        out=T64[:], in0=Ti[:, k * F:(k + 1) * F],
                        scalar1=64.0, scalar2=float(32 - 64 * c0),
                        op0=ALU.mult, op1=ALU.add)
                    dve.tensor_tensor(
                        out=U4[:, (g * 2 + (k % 2)) * F:(g * 2 + (k % 2) + 1) * F],
                        in0=T64[:], in1=L2[:, h * F:(h + 1) * F], op=ALU.subtract)
                ins = dve.engine_nop()
                ins.then_inc(s_u, 4)
                # ---- Q/M families at pair end ----
                if k % 2 == 1:
                    pair = k // 2
                    qi = 0
                    for g, (c0, sz) in enumerate(GROUPS):
                        ug = U4[:, g * 2 * F:(g * 2 + 2) * F]
                        for cp in range(sz + 1):
                            dve.tensor_scalar(
                                out=SCR[:, 0:2 * F].bitcast(f16), in0=ug,
                                scalar1=float(64 * cp), scalar2=None,
                                op0=ALU.max, op1=ALU.add,
                                accum_out=ACC[:, Q_OFF + pair * NQ + qi:Q_OFF + pair * NQ + qi + 1])
                            qi += 1
                    u0 = U4[:, 0:2 * F]
                    for c in range(NCLASS):
                        dve.tensor_scalar(
                            out=SCR[:, 0:2 * F].bitcast(f16), in0=u0,
                            scalar1=float(64 * c), scalar2=None,
                            op0=ALU.is_ge, op1=ALU.add,
                            accum_out=ACC[:, M_OFF + pair * NCLASS + c:M_OFF + pair * NCLASS + c + 1])
                    ins = dve.engine_nop()
                    ins.then_inc(s_qm, 1)

        @block.gpsimd
        def _(gp):
            gp.dma_start(Ti[:], target[:, :]).then_inc(s_t, 16)
            gp.wait_ge(s_t, 16)
            for k in range(NCHUNK):
                h = k % 2
                if k >= 2:
                    gp.wait_ge(s_aD, NDVE_A * (k - 1))  # Tb_h free
                gp.tensor_copy(Tb2[:, h * F:(h + 1) * F], Ti[:, k * F:(k + 1) * F])
                ins = gp.engine_nop()
                ins.then_inc(s_tb, 1)
                ins = gp.engine_nop()
                ins.then_inc(s_aG, NGP_A)

        @block.tensor
        def _(pe):
            pe.wait_ge(s_qm, NPAIR)
            pe.wait_ge(s_aD, NCHUNK * NDVE_A)
            pe.wait_ge(s_aG, NCHUNK * NGP_A)
            pe.matmul(ps[:], lhsT=ones[:], rhs=ACC[:], start=True, stop=True).then_inc(s_mm, 1)

    return nc


_CACHE = {}


def _get_nc():
    if "nc" not in _CACHE:
        _CACHE["nc"] = _build()
    return _CACHE["nc"]


def _run(logits, target, trace=False):
    nc = _get_nc()
    in_maps = []
    for i in range(NCORES):
        in_maps.append({
            "logits": np.ascontiguousarray(logits[i].reshape(NCLASS, P, COLS)),
            "target": np.ascontiguousarray(target[i].reshape(P, COLS)),
        })
    return run_bass_kernel_spmd(nc, in_maps, core_ids=list(range(NCORES)), trace=trace)


def _combine(results):
    NPIX_PAIR = float(NCORES * P * 2 * F)   # pixels per pair across cores
    A = np.zeros(NCLASS, np.float64)
    Q = np.zeros((NPAIR, NQ), np.float64)
    M = np.zeros(NCLASS + 1, np.float64)
    for i in range(NCORES):
        r = results[i]["out"].astype(np.float64).reshape(NCOL)
        a = r[A_OFF:A_OFF + NCHUNK * NCLASS].reshape(NCHUNK, NCLASS)
        A += a.sum(axis=0)
        Q += r[Q_OFF:Q_OFF + NPAIR * NQ].reshape(NPAIR, NQ)
        M[:NCLASS] += r[M_OFF:M_OFF + NPAIR * NCLASS].reshape(NPAIR, NCLASS).sum(axis=0)
    N = M[:NCLASS] - M[1:]
    B = np.zeros(NCLASS, np.float64)
    Qs = Q.sum(axis=0)          # summed over pairs (equations are linear)
    NPIX = NPIX_PAIR * NPAIR
    qi = 0
    for g, (c0, sz) in enumerate(GROUPS):
        qg = Qs[qi:qi + sz + 1]
        for cp in range(sz):
            c = c0 + cp
            D32 = qg[cp] - qg[cp + 1] + 64.0 * NPIX - 64.0 * M[c + 1]
            B[c] = 32.0 * N[c] - D32
        qi += sz + 1
    w = np.where(N > 0, (1.0 - BETA) / (1.0 - BETA ** N), 0.0)
    num = float((w * (A - B)).sum())
    den = float((w * N).sum())
    return np.float32(-num / den)


def kernel(logits, target):
    assert logits.shape == (NCORES, NCLASS, 512, 1024) and logits.dtype == np.float32
    assert target.shape == (NCORES, 512, 1024) and target.dtype == np.int32
    r = _run(logits, target, trace=False)
    return _combine(r.results)
